# revision 16
# baseline (speedup 1.0000x reference)
"""MACE node-message block on 8 Trainium2 NeuronCores.

Strategy (receiver-sharded, no collectives):
  - Host sorts edges by receiver node and assigns each of the 8 cores a
    contiguous range of 1250 receiver nodes (+ its incoming edges).
  - Host gathers sender features per edge (np.take), transposes layouts,
    pads each core's edges into NCHUNK node-chunks x TPC tiles of 128 edges,
    and builds per-tile "H" scatter matrices H[j][e, n] = y_j[e] * (rel[e]==n)
    that fold the edge spherical harmonics (y0, y1) AND the segment-sum into
    tensor-engine matmuls.
  - Device per tile (128 edges): up-project gathered feats (PE), radial MLP
    (PE + Silu on ACT), 8 elementwise tensor-product blocks (DVE, bf16),
    then 8 PE matmuls (lhsT=product block, rhs=H) accumulating messages
    transposed [c, n] in PSUM over a chunk. Per chunk: output linear (PE)
    and DMA of the [128 nodes, 512] output shard.
  - Host reassembles the full [10000, 512] output (including the m-minor
    interleave of the vector irreps).
"""

import numpy as np
import ml_dtypes

# ---- problem constants (hardcoded; kernel.py must be self-contained) ----
N_NODES = 10000
E_EDGES = 160000
C = 128
RB = 8
HID = 64
AVG_NEIGH = 16.0

C_000 = float(np.sqrt(0.5))
C_110 = float(np.sqrt(0.5) / np.sqrt(3.0))
C_011 = float(np.sqrt(1.5) / np.sqrt(3.0))
C_101 = float(np.sqrt(1.5) / np.sqrt(3.0))

NCORES = 8
NODES_PER_CORE = N_NODES // NCORES  # 1250
NCHUNK = 10            # node-chunks per core (<=128 nodes each)
TPC = 16               # tiles of 128 edges per chunk
CHUNK_SLOTS = TPC * 128   # 2048 edge slots per chunk
EPAD = NCHUNK * CHUNK_SLOTS
NTILES = NCHUNK * TPC
SUPER = 4              # tiles per radial-MLP supertile (512 edges)

BF16 = ml_dtypes.bfloat16


# --------------------------------------------------------------------------
# Host-side sharding / layout preparation
# --------------------------------------------------------------------------

def _host_prep(node_feats, edge_attrs, edge_feats, edge_index):
    sender = edge_index[0].astype(np.int64)
    receiver = edge_index[1].astype(np.int64)
    deg = np.bincount(receiver, minlength=N_NODES)
    order = np.argsort(receiver, kind="stable")
    recv_sorted = receiver[order]
    # index of first edge (in sorted order) for each node
    node_edge_start = np.concatenate([[0], np.cumsum(deg)])

    per_core = []
    for c in range(NCORES):
        lo, hi = NODES_PER_CORE * c, NODES_PER_CORE * (c + 1)
        # greedy chunking: <=128 nodes and <=CHUNK_SLOTS edges per chunk
        chunks = []  # (node_start, node_end, edge_count)
        n = lo
        while n < hi:
            start = n
            ec = 0
            while n < hi and (n - start) < 128 and ec + deg[n] <= CHUNK_SLOTS:
                ec += deg[n]
                n += 1
            chunks.append((start, n, int(ec)))
        assert len(chunks) <= NCHUNK, (
            f"core {c}: needs {len(chunks)} chunks > NCHUNK={NCHUNK}"
        )
        while len(chunks) < NCHUNK:
            chunks.append((hi, hi, 0))

        slot_sender = np.zeros(EPAD, np.int64)
        slot_rel = np.zeros(EPAD, np.int64)
        slot_y = np.zeros((EPAD, 4), np.float32)
        slot_ef = np.zeros((EPAD, RB), np.float32)
        for k, (s, e, ec) in enumerate(chunks):
            if ec == 0:
                continue
            seg = order[node_edge_start[s]: node_edge_start[s] + ec]
            base = k * CHUNK_SLOTS
            slot_sender[base: base + ec] = sender[seg]
            slot_rel[base: base + ec] = receiver[seg] - s
            slot_y[base: base + ec] = edge_attrs[seg]
            slot_ef[base: base + ec] = edge_feats[seg]

        # gathered sender feats, transposed per block: [t, cin, blk, e]
        g = node_feats[slot_sender]                      # [EPAD, 512]
        s_blk = g[:, :C]
        v = g[:, C:].reshape(EPAD, C, 3)
        blocks = np.stack([s_blk, v[:, :, 0], v[:, :, 1], v[:, :, 2]], axis=1)
        gfeat = np.ascontiguousarray(
            blocks.reshape(NTILES, 128, 4, C).transpose(0, 3, 2, 1)
        ).astype(BF16)

        # H matrices: hmat[t, e, j, n] = y_j * onehot(rel), j=4: plain
        # onehot (zero on pad slots)
        hm = np.zeros((NTILES, 128, 5, 128), np.float32)
        tidx = np.arange(EPAD) // 128
        eidx = np.arange(EPAD) % 128
        for j in range(4):
            hm[tidx, eidx, j, slot_rel] = slot_y[:, j]
        valid = np.zeros(EPAD, np.float32)
        for k, (s, e, ec) in enumerate(chunks):
            valid[k * CHUNK_SLOTS: k * CHUNK_SLOTS + ec] = 1.0
        hm[tidx, eidx, 4, slot_rel] = valid
        hmat = hm.reshape(NTILES, 128, 640).astype(BF16)
        # per-edge y1 scalars (f32) for the DVE presum of the p3 path
        yatt = np.ascontiguousarray(
            slot_y[:, 1:4].reshape(NTILES, 128, 3)).astype(np.float32)

        # radial basis, transposed per chunk: [k, r, slot]
        eft = np.ascontiguousarray(
            slot_ef.reshape(NCHUNK, CHUNK_SLOTS, RB).transpose(0, 2, 1)
        ).astype(BF16)

        g4 = np.ascontiguousarray(
            gfeat.reshape(NTILES // SUPER, SUPER, 128, 512)
            .transpose(0, 2, 1, 3))
        h4 = np.ascontiguousarray(
            hmat.reshape(NTILES // SUPER, SUPER, 128, 640)
            .transpose(0, 2, 1, 3))
        y4 = np.ascontiguousarray(
            yatt.reshape(NTILES // SUPER, SUPER, 128, 3)
            .transpose(0, 2, 1, 3))
        per_core.append({
            "chunks": chunks,
            "gfeat": g4,
            "hmat": h4,
            "yatt": y4,
            "eft": eft,
        })
    return per_core


def _weights_prep(W_up_s, W_up_v, W_mlp1, W_mlp2, W_mlp3, W_mlp4,
                  W_lin_s, W_lin_v):
    su = 1.0 / np.sqrt(np.float32(C))
    wup = np.stack([W_up_s * su, W_up_v * su], axis=1).astype(BF16)  # [cin,2,cout]

    w1 = (W_mlp1 / np.sqrt(np.float32(RB))).astype(BF16)
    w2 = (W_mlp2 / np.sqrt(np.float32(HID))).astype(BF16)
    w3 = (W_mlp3 / np.sqrt(np.float32(HID))).astype(BF16)
    # per-path constants folded into the last MLP layer (tpw order 0,1,2,3)
    scales = np.array([C_000, C_011, C_101, C_110], np.float32)
    w4p = ((W_mlp4 / np.sqrt(np.float32(HID))).reshape(HID, 4, C)
           * scales[None, :, None]).reshape(HID, 4 * C).astype(BF16)

    sl = 1.0 / (np.sqrt(np.float32(2 * C)) * AVG_NEIGH)
    wls = W_lin_s * sl   # [256, 128]
    wlv = W_lin_v * sl
    wl = np.stack([wls[:C], wls[C:], wlv[:C], wlv[C:]], axis=1).astype(np.float32)
    # wl[cin, j, d]: j=0 Wls rows 0:128, j=1 Wls rows 128:256, j=2/3 Wlv
    return {"wup": wup, "w1": w1, "w2": w2, "w3": w3, "w4p": w4p, "wl": wl}


# --------------------------------------------------------------------------
# Device program
# --------------------------------------------------------------------------

def build_program():
    import concourse.bacc as bacc
    import concourse.mybir as mybir
    import concourse.tile as tile

    f32 = mybir.dt.float32
    bf16 = mybir.dt.bfloat16
    MUL = mybir.AluOpType.mult
    ADD = mybir.AluOpType.add

    nc = bacc.Bacc(None, target_bir_lowering=False)

    NSUPER = NTILES // SUPER
    gfeatD = nc.dram_tensor("gfeat", [NSUPER, 128, SUPER, 512], bf16,
                            kind="ExternalInput")
    hmatD = nc.dram_tensor("hmat", [NSUPER, 128, SUPER, 640], bf16,
                           kind="ExternalInput")
    yattD = nc.dram_tensor("yatt", [NSUPER, 128, SUPER, 3], f32,
                           kind="ExternalInput")
    eftD = nc.dram_tensor("eft", [NCHUNK, RB, CHUNK_SLOTS], bf16,
                          kind="ExternalInput")
    wupD = nc.dram_tensor("wup", [128, 2, 128], bf16, kind="ExternalInput")
    w1D = nc.dram_tensor("w1", [RB, HID], bf16, kind="ExternalInput")
    w2D = nc.dram_tensor("w2", [HID, HID], bf16, kind="ExternalInput")
    w3D = nc.dram_tensor("w3", [HID, HID], bf16, kind="ExternalInput")
    w4pD = nc.dram_tensor("w4p", [HID, 4 * C], bf16, kind="ExternalInput")
    wlD = nc.dram_tensor("wl", [128, 4, 128], f32, kind="ExternalInput")
    outD = nc.dram_tensor("outb", [NCHUNK * 128, 512], f32,
                          kind="ExternalOutput")

    SILU = mybir.ActivationFunctionType.Silu
    COPYF = mybir.ActivationFunctionType.Copy
    from concourse.masks import make_identity

    with tile.TileContext(nc) as tc:
        with (
            tc.tile_pool(name="const", bufs=1) as cp,
            tc.tile_pool(name="work", bufs=4) as wp,
            tc.tile_pool(name="hs", bufs=3) as hsp,
            tc.tile_pool(name="chk", bufs=2) as chp,
            tc.tile_pool(name="ps", bufs=2, space="PSUM") as ps,
            tc.tile_pool(name="msgp", bufs=1, space="PSUM") as msgp,
        ):
            # constants
            wupS = cp.tile([128, 2, 128], bf16, tag="wup")
            w1S = cp.tile([RB, HID], bf16, tag="w1")
            w2S = cp.tile([HID, HID], bf16, tag="w2")
            w3S = cp.tile([HID, HID], bf16, tag="w3")
            w4pS = cp.tile([HID, 4 * C], bf16, tag="w4p")
            wlS = cp.tile([128, 4, 128], f32, tag="wl")
            nc.sync.dma_start(out=wupS[:], in_=wupD[:])
            nc.sync.dma_start(out=w1S[:], in_=w1D[:])
            nc.sync.dma_start(out=w2S[:], in_=w2D[:])
            nc.sync.dma_start(out=w3S[:], in_=w3D[:])
            nc.sync.dma_start(out=w4pS[:], in_=w4pD[:])
            nc.sync.dma_start(out=wlS[:], in_=wlD[:])

            for k in range(NCHUNK):
                efS = chp.tile([RB, CHUNK_SLOTS], bf16, tag="ef")
                nc.sync.dma_start(out=efS[:], in_=eftD[k])

                # message accumulators, [cin-block, n] orientation:
                # msgA = (vx1|vy1|vz1|s1), msgB = (vx2|vy2|vz2|s2)
                msgA = msgp.tile([128, 512], f32, tag="msgA")
                msgB = msgp.tile([128, 512], f32, tag="msgB")

                for sidx in range(TPC // SUPER):
                    sg = k * (TPC // SUPER) + sidx
                    g4 = wp.tile([128, SUPER, 512], bf16, tag="g")
                    nc.sync.dma_start(out=g4[:], in_=gfeatD[sg])
                    h4 = wp.tile([128, SUPER, 640], bf16, tag="hm")
                    nc.sync.dma_start(out=h4[:], in_=hmatD[sg])
                    y4 = wp.tile([128, SUPER, 3], f32, tag="ya")
                    nc.sync.dma_start(out=y4[:], in_=yattD[sg])

                    # ---- radial MLP over a supertile of 512 edges ----
                    esl = slice(sidx * SUPER * 128, (sidx + 1) * SUPER * 128)
                    h1p = ps.tile([HID, SUPER * 128], f32, tag="ht")
                    nc.tensor.matmul(out=h1p[:], lhsT=w1S[:], rhs=efS[:, esl],
                                     start=True, stop=True)
                    h1s = hsp.tile([HID, SUPER * 128], bf16, tag="h1s")
                    nc.scalar.activation(out=h1s[:], in_=h1p[:], func=SILU)
                    h2p = ps.tile([HID, SUPER * 128], f32, tag="ht")
                    nc.tensor.matmul(out=h2p[:], lhsT=w2S[:], rhs=h1s[:],
                                     start=True, stop=True)
                    h2s = hsp.tile([HID, SUPER * 128], bf16, tag="h2s")
                    nc.scalar.activation(out=h2s[:], in_=h2p[:], func=SILU)
                    h3p = ps.tile([HID, SUPER * 128], f32, tag="ht")
                    nc.tensor.matmul(out=h3p[:], lhsT=w3S[:], rhs=h2s[:],
                                     start=True, stop=True)
                    h3s = hsp.tile([HID, SUPER * 128], bf16, tag="h3s")
                    nc.scalar.activation(out=h3s[:], in_=h3p[:], func=SILU)

                    for u in range(SUPER):
                        tl = sidx * SUPER + u          # tile index in chunk
                        t = k * TPC + tl               # global tile index
                        first = tl == 0
                        last = tl == TPC - 1

                        gS = g4[:, u, :]
                        hS = h4[:, u, :]

                        # ---- up-projection + tpw into one 2-bank tile ----
                        # bank 1: feat blocks (se|vx|vy|vz); bank 2: tpw.
                        # PSUM: start=True clears has_written for the WHOLE
                        # bank -> exactly one start/stop per bank lifetime.
                        ftP = ps.tile([128, 1024], f32, tag="ftp")
                        for b in range(4):
                            nc.tensor.matmul(
                                out=ftP[:, b * 128:(b + 1) * 128],
                                lhsT=gS[:, b * 128:(b + 1) * 128],
                                rhs=wupS[:, min(b, 1), :],
                                start=b == 0, stop=b == 3)
                        nc.tensor.matmul(
                            out=ftP[:, 512:1024],
                            lhsT=h3s[:, u * 128:(u + 1) * 128],
                            rhs=w4pS[:], start=True, stop=True)
                        # one cast PSUM->SBUF: blocks 0-3 feat, 4-7 tpw
                        ftS = wp.tile([128, 8, 128], bf16, tag="ft")
                        nc.scalar.activation(out=ftS[:], in_=ftP[:],
                                             func=COPYF)
                        featS = ftS[:, 0:4, :]
                        tpwS = ftS[:, 4:8, :]

                        # ---- elementwise product blocks (DVE, bf16) ----
                        # slots: p0, p2x, p2y, p2z, p1, p3x, p3y, p3z, r, t
                        prodS = wp.tile([128, 10, 128], bf16, tag="prod")
                        nc.vector.tensor_tensor(
                            out=prodS[:, 0, :], in0=tpwS[:, 0, :],
                            in1=featS[:, 0, :], op=MUL)
                        nc.vector.tensor_tensor(
                            out=prodS[:, 1:4, :],
                            in0=tpwS[:, 2:3, :].broadcast_to([128, 3, 128]),
                            in1=featS[:, 1:4, :], op=MUL)
                        nc.vector.tensor_tensor(
                            out=prodS[:, 4, :], in0=tpwS[:, 1, :],
                            in1=featS[:, 0, :], op=MUL)
                        nc.vector.tensor_tensor(
                            out=prodS[:, 5:8, :],
                            in0=tpwS[:, 3:4, :].broadcast_to([128, 3, 128]),
                            in1=featS[:, 1:4, :], op=MUL)
                        # presum the p3 path: r = sum_m y1m * p3m (f32 y)
                        ysl = y4[:, u, :]
                        nc.vector.tensor_scalar_mul(
                            prodS[:, 9, :], prodS[:, 5, :], ysl[:, 0:1])
                        nc.vector.scalar_tensor_tensor(
                            out=prodS[:, 8, :], in0=prodS[:, 6, :],
                            scalar=ysl[:, 1:2], in1=prodS[:, 9, :],
                            op0=MUL, op1=ADD)
                        nc.vector.scalar_tensor_tensor(
                            out=prodS[:, 9, :], in0=prodS[:, 7, :],
                            scalar=ysl[:, 2:3], in1=prodS[:, 8, :],
                            op0=MUL, op1=ADD)

                        # ---- weighted segment-sum (product stationary) ----
                        # out orientation [cin-block, n]
                        nc.tensor.matmul(out=msgA[:, 0:384],
                                         lhsT=prodS[:, 4, :],
                                         rhs=hS[:, 128:512],
                                         start=first, stop=False)
                        nc.tensor.matmul(out=msgA[:, 384:512],
                                         lhsT=prodS[:, 0, :],
                                         rhs=hS[:, 0:128],
                                         start=False, stop=last)
                        for m in range(3):
                            nc.tensor.matmul(
                                out=msgB[:, m * 128:(m + 1) * 128],
                                lhsT=prodS[:, 1 + m, :],
                                rhs=hS[:, 0:128],
                                start=first and m == 0, stop=False)
                        nc.tensor.matmul(out=msgB[:, 384:512],
                                         lhsT=prodS[:, 9, :],
                                         rhs=hS[:, 512:640],
                                         start=False, stop=last)

                # ---- chunk epilogue: output linear + store ----
                msgSA = chp.tile([128, 512], f32, tag="msgSA")
                nc.vector.tensor_copy(out=msgSA[:], in_=msgA[:])
                msgSB = chp.tile([128, 512], f32, tag="msgSB")
                nc.vector.tensor_copy(out=msgSB[:], in_=msgB[:])

                outPF = ps.tile([128, 1024], f32, tag="ftp")
                outP = outPF[:, 0:512]
                nc.tensor.matmul(out=outP[:, 0:128], lhsT=msgSA[:, 384:512],
                                 rhs=wlS[:, 0, :], start=True, stop=False)
                nc.tensor.matmul(out=outP[:, 0:128], lhsT=msgSB[:, 384:512],
                                 rhs=wlS[:, 1, :], start=False, stop=False)
                for m in range(3):
                    osl = slice((1 + m) * 128, (2 + m) * 128)
                    nc.tensor.matmul(out=outP[:, osl],
                                     lhsT=msgSA[:, m * 128:(m + 1) * 128],
                                     rhs=wlS[:, 2, :], start=False, stop=False)
                    nc.tensor.matmul(out=outP[:, osl],
                                     lhsT=msgSB[:, m * 128:(m + 1) * 128],
                                     rhs=wlS[:, 3, :], start=False,
                                     stop=m == 2)
                outS = chp.tile([128, 512], f32, tag="outS")
                nc.vector.tensor_copy(out=outS[:], in_=outP[:])
                nc.sync.dma_start(out=outD[k * 128:(k + 1) * 128, :],
                                  in_=outS[:])

    nc.compile()
    return nc


# --------------------------------------------------------------------------
# Entry point
# --------------------------------------------------------------------------

def _assemble(results, per_core):
    out = np.zeros((N_NODES, 512), np.float32)
    for c in range(NCORES):
        ob = results[c]["outb"]
        for k, (s, e, _ec) in enumerate(per_core[c]["chunks"]):
            w = e - s
            if w == 0:
                continue
            rows = ob[k * 128: k * 128 + w]
            out[s:e, :C] = rows[:, :C]
            out[s:e, C:] = np.stack(
                [rows[:, C:2 * C], rows[:, 2 * C:3 * C], rows[:, 3 * C:]],
                axis=2).reshape(w, 3 * C)
    return out


def run(inputs, trace=False, **kwargs):
    from concourse.bass_utils import run_bass_kernel_spmd

    per_core = _host_prep(inputs["node_feats"], inputs["edge_attrs"],
                          inputs["edge_feats"], inputs["edge_index"])
    wts = _weights_prep(inputs["W_up_s"], inputs["W_up_v"], inputs["W_mlp1"],
                        inputs["W_mlp2"], inputs["W_mlp3"], inputs["W_mlp4"],
                        inputs["W_lin_s"], inputs["W_lin_v"])
    in_maps = [
        {"gfeat": pc["gfeat"], "hmat": pc["hmat"], "yatt": pc["yatt"],
         "eft": pc["eft"], **wts}
        for pc in per_core
    ]
    nc = build_program()
    res = run_bass_kernel_spmd(nc, in_maps, core_ids=list(range(NCORES)),
                               trace=trace, **kwargs)
    return _assemble(res.results, per_core), res


def kernel(**inputs):
    return run(inputs)[0]


if __name__ == "__main__":
    # smoke: host prep only
    rng = np.random.default_rng(0)
    ins = {
        "node_feats": rng.standard_normal((N_NODES, 512), np.float32),
        "edge_attrs": rng.standard_normal((E_EDGES, 4), np.float32),
        "edge_feats": rng.standard_normal((E_EDGES, RB), np.float32),
        "edge_index": rng.integers(0, N_NODES, (2, E_EDGES)).astype(np.int32),
    }
    pc = _host_prep(ins["node_feats"], ins["edge_attrs"], ins["edge_feats"],
                    ins["edge_index"])
    for c, d in enumerate(pc):
        used = [ch for ch in d["chunks"] if ch[2] > 0]
        print(f"core {c}: {len(used)} chunks used, "
              f"edges={sum(ch[2] for ch in d['chunks'])}")


# revision 17
# speedup vs baseline: 1.0786x; 1.0786x over previous
"""MACE node-message block on 8 Trainium2 NeuronCores.

Strategy (receiver-sharded, no collectives):
  - Host sorts edges by receiver node and assigns each of the 8 cores a
    contiguous range of 1250 receiver nodes (+ its incoming edges).
  - Host gathers sender features per edge (np.take), transposes layouts,
    pads each core's edges into NCHUNK node-chunks x TPC tiles of 128 edges,
    and builds per-tile "H" scatter matrices H[j][e, n] = y_j[e] * (rel[e]==n)
    that fold the edge spherical harmonics (y0, y1) AND the segment-sum into
    tensor-engine matmuls.
  - Device per tile (128 edges): up-project gathered feats (PE), radial MLP
    (PE + Silu on ACT), 8 elementwise tensor-product blocks (DVE, bf16),
    then 8 PE matmuls (lhsT=product block, rhs=H) accumulating messages
    transposed [c, n] in PSUM over a chunk. Per chunk: output linear (PE)
    and DMA of the [128 nodes, 512] output shard.
  - Host reassembles the full [10000, 512] output (including the m-minor
    interleave of the vector irreps).
"""

import numpy as np
import ml_dtypes

# ---- problem constants (hardcoded; kernel.py must be self-contained) ----
N_NODES = 10000
E_EDGES = 160000
C = 128
RB = 8
HID = 64
AVG_NEIGH = 16.0

C_000 = float(np.sqrt(0.5))
C_110 = float(np.sqrt(0.5) / np.sqrt(3.0))
C_011 = float(np.sqrt(1.5) / np.sqrt(3.0))
C_101 = float(np.sqrt(1.5) / np.sqrt(3.0))

NCORES = 8
NODES_PER_CORE = N_NODES // NCORES  # 1250
NCHUNK = 10            # node-chunks per core (<=128 nodes each)
TPC = 16               # tiles of 128 edges per chunk
CHUNK_SLOTS = TPC * 128   # 2048 edge slots per chunk
EPAD = NCHUNK * CHUNK_SLOTS
NTILES = NCHUNK * TPC
SUPER = 4              # tiles per radial-MLP supertile (512 edges)

BF16 = ml_dtypes.bfloat16


# --------------------------------------------------------------------------
# Host-side sharding / layout preparation
# --------------------------------------------------------------------------

def _host_prep(node_feats, edge_attrs, edge_feats, edge_index):
    sender = edge_index[0].astype(np.int64)
    receiver = edge_index[1].astype(np.int64)
    deg = np.bincount(receiver, minlength=N_NODES)
    order = np.argsort(receiver, kind="stable")
    recv_sorted = receiver[order]
    # index of first edge (in sorted order) for each node
    node_edge_start = np.concatenate([[0], np.cumsum(deg)])

    per_core = []
    for c in range(NCORES):
        lo, hi = NODES_PER_CORE * c, NODES_PER_CORE * (c + 1)
        # greedy chunking: <=128 nodes and <=CHUNK_SLOTS edges per chunk
        chunks = []  # (node_start, node_end, edge_count)
        n = lo
        while n < hi:
            start = n
            ec = 0
            while n < hi and (n - start) < 128 and ec + deg[n] <= CHUNK_SLOTS:
                ec += deg[n]
                n += 1
            chunks.append((start, n, int(ec)))
        assert len(chunks) <= NCHUNK, (
            f"core {c}: needs {len(chunks)} chunks > NCHUNK={NCHUNK}"
        )
        while len(chunks) < NCHUNK:
            chunks.append((hi, hi, 0))

        slot_sender = np.zeros(EPAD, np.int64)
        slot_rel = np.zeros(EPAD, np.int64)
        slot_y = np.zeros((EPAD, 4), np.float32)
        slot_ef = np.zeros((EPAD, RB), np.float32)
        for k, (s, e, ec) in enumerate(chunks):
            if ec == 0:
                continue
            seg = order[node_edge_start[s]: node_edge_start[s] + ec]
            base = k * CHUNK_SLOTS
            slot_sender[base: base + ec] = sender[seg]
            slot_rel[base: base + ec] = receiver[seg] - s
            slot_y[base: base + ec] = edge_attrs[seg]
            slot_ef[base: base + ec] = edge_feats[seg]

        # gathered sender feats, transposed per block: [t, cin, blk, e]
        g = node_feats[slot_sender]                      # [EPAD, 512]
        s_blk = g[:, :C]
        v = g[:, C:].reshape(EPAD, C, 3)
        blocks = np.stack([s_blk, v[:, :, 0], v[:, :, 1], v[:, :, 2]], axis=1)
        gfeat = np.ascontiguousarray(
            blocks.reshape(NTILES, 128, 4, C).transpose(0, 3, 2, 1)
        ).astype(BF16)

        # H matrices: hmat[t, e, j, n] = y_j * onehot(rel)
        hm = np.zeros((NTILES, 128, 4, 128), np.float32)
        tidx = np.arange(EPAD) // 128
        eidx = np.arange(EPAD) % 128
        for j in range(4):
            hm[tidx, eidx, j, slot_rel] = slot_y[:, j]
        hmat = hm.reshape(NTILES, 128, 512).astype(BF16)

        # radial basis, transposed per chunk: [k, r, slot]
        eft = np.ascontiguousarray(
            slot_ef.reshape(NCHUNK, CHUNK_SLOTS, RB).transpose(0, 2, 1)
        ).astype(BF16)

        g4 = np.ascontiguousarray(
            gfeat.reshape(NTILES // SUPER, SUPER, 128, 512)
            .transpose(0, 2, 1, 3))
        h4 = np.ascontiguousarray(
            hmat.reshape(NTILES // SUPER, SUPER, 128, 512)
            .transpose(0, 2, 1, 3))
        per_core.append({
            "chunks": chunks,
            "gfeat": g4,
            "hmat": h4,
            "eft": eft,
        })
    return per_core


def _weights_prep(W_up_s, W_up_v, W_mlp1, W_mlp2, W_mlp3, W_mlp4,
                  W_lin_s, W_lin_v):
    su = 1.0 / np.sqrt(np.float32(C))
    wup = np.stack([W_up_s * su, W_up_v * su], axis=1).astype(BF16)  # [cin,2,cout]

    w1 = (W_mlp1 / np.sqrt(np.float32(RB))).astype(BF16)
    w2 = (W_mlp2 / np.sqrt(np.float32(HID))).astype(BF16)
    w3 = (W_mlp3 / np.sqrt(np.float32(HID))).astype(BF16)
    # per-path constants folded into the last MLP layer (tpw order 0,1,2,3)
    scales = np.array([C_000, C_011, C_101, C_110], np.float32)
    w4p = ((W_mlp4 / np.sqrt(np.float32(HID))).reshape(HID, 4, C)
           * scales[None, :, None]).reshape(HID, 4 * C).astype(BF16)

    sl = 1.0 / (np.sqrt(np.float32(2 * C)) * AVG_NEIGH)
    wls = W_lin_s * sl   # [256, 128]
    wlv = W_lin_v * sl
    wl = np.stack([wls[:C], wls[C:], wlv[:C], wlv[C:]], axis=1).astype(np.float32)
    # wl[cin, j, d]: j=0 Wls rows 0:128, j=1 Wls rows 128:256, j=2/3 Wlv
    return {"wup": wup, "w1": w1, "w2": w2, "w3": w3, "w4p": w4p, "wl": wl}


# --------------------------------------------------------------------------
# Device program
# --------------------------------------------------------------------------

def build_program():
    import concourse.bacc as bacc
    import concourse.mybir as mybir
    import concourse.tile as tile

    f32 = mybir.dt.float32
    bf16 = mybir.dt.bfloat16
    MUL = mybir.AluOpType.mult
    ADD = mybir.AluOpType.add

    nc = bacc.Bacc(None, target_bir_lowering=False)

    NSUPER = NTILES // SUPER
    gfeatD = nc.dram_tensor("gfeat", [NSUPER, 128, SUPER, 512], bf16,
                            kind="ExternalInput")
    hmatD = nc.dram_tensor("hmat", [NSUPER, 128, SUPER, 512], bf16,
                           kind="ExternalInput")
    eftD = nc.dram_tensor("eft", [NCHUNK, RB, CHUNK_SLOTS], bf16,
                          kind="ExternalInput")
    wupD = nc.dram_tensor("wup", [128, 2, 128], bf16, kind="ExternalInput")
    w1D = nc.dram_tensor("w1", [RB, HID], bf16, kind="ExternalInput")
    w2D = nc.dram_tensor("w2", [HID, HID], bf16, kind="ExternalInput")
    w3D = nc.dram_tensor("w3", [HID, HID], bf16, kind="ExternalInput")
    w4pD = nc.dram_tensor("w4p", [HID, 4 * C], bf16, kind="ExternalInput")
    wlD = nc.dram_tensor("wl", [128, 4, 128], f32, kind="ExternalInput")
    outD = nc.dram_tensor("outb", [NCHUNK * 128, 512], f32,
                          kind="ExternalOutput")

    SILU = mybir.ActivationFunctionType.Silu
    COPYF = mybir.ActivationFunctionType.Copy
    from concourse.masks import make_identity

    with tile.TileContext(nc) as tc:
        with (
            tc.tile_pool(name="const", bufs=1) as cp,
            tc.tile_pool(name="work", bufs=4) as wp,
            tc.tile_pool(name="hs", bufs=3) as hsp,
            tc.tile_pool(name="chk", bufs=2) as chp,
            tc.tile_pool(name="ps", bufs=2, space="PSUM") as ps,
            tc.tile_pool(name="msgp", bufs=1, space="PSUM") as msgp,
        ):
            # constants
            wupS = cp.tile([128, 2, 128], bf16, tag="wup")
            w1S = cp.tile([RB, HID], bf16, tag="w1")
            w2S = cp.tile([HID, HID], bf16, tag="w2")
            w3S = cp.tile([HID, HID], bf16, tag="w3")
            w4pS = cp.tile([HID, 4 * C], bf16, tag="w4p")
            wlS = cp.tile([128, 4, 128], f32, tag="wl")
            nc.sync.dma_start(out=wupS[:], in_=wupD[:])
            nc.sync.dma_start(out=w1S[:], in_=w1D[:])
            nc.sync.dma_start(out=w2S[:], in_=w2D[:])
            nc.sync.dma_start(out=w3S[:], in_=w3D[:])
            nc.sync.dma_start(out=w4pS[:], in_=w4pD[:])
            nc.sync.dma_start(out=wlS[:], in_=wlD[:])

            for k in range(NCHUNK):
                efS = chp.tile([RB, CHUNK_SLOTS], bf16, tag="ef")
                nc.sync.dma_start(out=efS[:], in_=eftD[k])

                # message accumulators, [cin-block, n] orientation:
                # msgA = (vx1|vy1|vz1|s1), msgB = (vx2|vy2|vz2|s2)
                msgA = msgp.tile([128, 512], f32, tag="msgA")
                msgB = msgp.tile([128, 512], f32, tag="msgB")

                for sidx in range(TPC // SUPER):
                    sg = k * (TPC // SUPER) + sidx
                    g4 = wp.tile([128, SUPER, 512], bf16, tag="g")
                    nc.sync.dma_start(out=g4[:], in_=gfeatD[sg])
                    h4 = wp.tile([128, SUPER, 512], bf16, tag="hm")
                    nc.sync.dma_start(out=h4[:], in_=hmatD[sg])

                    # ---- radial MLP over a supertile of 512 edges ----
                    esl = slice(sidx * SUPER * 128, (sidx + 1) * SUPER * 128)
                    h1p = ps.tile([HID, SUPER * 128], f32, tag="ht")
                    nc.tensor.matmul(out=h1p[:], lhsT=w1S[:], rhs=efS[:, esl],
                                     start=True, stop=True)
                    h1s = hsp.tile([HID, SUPER * 128], bf16, tag="h1s")
                    nc.scalar.activation(out=h1s[:], in_=h1p[:], func=SILU)
                    h2p = ps.tile([HID, SUPER * 128], f32, tag="ht")
                    nc.tensor.matmul(out=h2p[:], lhsT=w2S[:], rhs=h1s[:],
                                     start=True, stop=True)
                    h2s = hsp.tile([HID, SUPER * 128], bf16, tag="h2s")
                    nc.scalar.activation(out=h2s[:], in_=h2p[:], func=SILU)
                    h3p = ps.tile([HID, SUPER * 128], f32, tag="ht")
                    nc.tensor.matmul(out=h3p[:], lhsT=w3S[:], rhs=h2s[:],
                                     start=True, stop=True)
                    h3s = hsp.tile([HID, SUPER * 128], bf16, tag="h3s")
                    nc.scalar.activation(out=h3s[:], in_=h3p[:], func=SILU)

                    for u in range(SUPER):
                        tl = sidx * SUPER + u          # tile index in chunk
                        t = k * TPC + tl               # global tile index
                        first = tl == 0
                        last = tl == TPC - 1

                        gS = g4[:, u, :]
                        hS = h4[:, u, :]

                        # ---- up-projection + tpw into one 2-bank tile ----
                        # bank 1: feat blocks (se|vx|vy|vz); bank 2: tpw.
                        # PSUM: start=True clears has_written for the WHOLE
                        # bank -> exactly one start/stop per bank lifetime.
                        ftP = ps.tile([128, 1024], f32, tag="ftp")
                        for b in range(4):
                            nc.tensor.matmul(
                                out=ftP[:, b * 128:(b + 1) * 128],
                                lhsT=gS[:, b * 128:(b + 1) * 128],
                                rhs=wupS[:, min(b, 1), :],
                                start=b == 0, stop=b == 3)
                        nc.tensor.matmul(
                            out=ftP[:, 512:1024],
                            lhsT=h3s[:, u * 128:(u + 1) * 128],
                            rhs=w4pS[:], start=True, stop=True)
                        # one cast PSUM->SBUF: blocks 0-3 feat, 4-7 tpw
                        ftS = wp.tile([128, 8, 128], bf16, tag="ft")
                        nc.scalar.activation(out=ftS[:], in_=ftP[:],
                                             func=COPYF)
                        featS = ftS[:, 0:4, :]
                        tpwS = ftS[:, 4:8, :]

                        # ---- elementwise product blocks (DVE, bf16) ----
                        # slots: p0, p2x, p2y, p2z, p1, p3x, p3y, p3z
                        prodS = wp.tile([128, 8, 128], bf16, tag="prod")
                        nc.vector.tensor_tensor(
                            out=prodS[:, 0, :], in0=tpwS[:, 0, :],
                            in1=featS[:, 0, :], op=MUL)
                        nc.vector.tensor_tensor(
                            out=prodS[:, 1:4, :],
                            in0=tpwS[:, 2:3, :].broadcast_to([128, 3, 128]),
                            in1=featS[:, 1:4, :], op=MUL)
                        nc.vector.tensor_tensor(
                            out=prodS[:, 4, :], in0=tpwS[:, 1, :],
                            in1=featS[:, 0, :], op=MUL)
                        nc.vector.tensor_tensor(
                            out=prodS[:, 5:8, :],
                            in0=tpwS[:, 3:4, :].broadcast_to([128, 3, 128]),
                            in1=featS[:, 1:4, :], op=MUL)
                        # ---- weighted segment-sum (product stationary) ----
                        # out orientation [cin-block, n]
                        nc.tensor.matmul(out=msgA[:, 0:384],
                                         lhsT=prodS[:, 4, :],
                                         rhs=hS[:, 128:512],
                                         start=first, stop=False)
                        nc.tensor.matmul(out=msgA[:, 384:512],
                                         lhsT=prodS[:, 0, :],
                                         rhs=hS[:, 0:128],
                                         start=False, stop=last)
                        for m in range(3):
                            nc.tensor.matmul(
                                out=msgB[:, m * 128:(m + 1) * 128],
                                lhsT=prodS[:, 1 + m, :],
                                rhs=hS[:, 0:128],
                                start=first and m == 0, stop=False)
                        for m in range(3):
                            nc.tensor.matmul(
                                out=msgB[:, 384:512],
                                lhsT=prodS[:, 5 + m, :],
                                rhs=hS[:, 128 + m * 128:256 + m * 128],
                                start=False,
                                stop=last and m == 2)

                # ---- chunk epilogue: output linear + store ----
                msgSA = chp.tile([128, 512], f32, tag="msgSA")
                nc.vector.tensor_copy(out=msgSA[:], in_=msgA[:])
                msgSB = chp.tile([128, 512], f32, tag="msgSB")
                nc.vector.tensor_copy(out=msgSB[:], in_=msgB[:])

                outPF = ps.tile([128, 1024], f32, tag="ftp")
                outP = outPF[:, 0:512]
                nc.tensor.matmul(out=outP[:, 0:128], lhsT=msgSA[:, 384:512],
                                 rhs=wlS[:, 0, :], start=True, stop=False)
                nc.tensor.matmul(out=outP[:, 0:128], lhsT=msgSB[:, 384:512],
                                 rhs=wlS[:, 1, :], start=False, stop=False)
                for m in range(3):
                    osl = slice((1 + m) * 128, (2 + m) * 128)
                    nc.tensor.matmul(out=outP[:, osl],
                                     lhsT=msgSA[:, m * 128:(m + 1) * 128],
                                     rhs=wlS[:, 2, :], start=False, stop=False)
                    nc.tensor.matmul(out=outP[:, osl],
                                     lhsT=msgSB[:, m * 128:(m + 1) * 128],
                                     rhs=wlS[:, 3, :], start=False,
                                     stop=m == 2)
                outS = chp.tile([128, 512], f32, tag="outS")
                nc.vector.tensor_copy(out=outS[:], in_=outP[:])
                nc.sync.dma_start(out=outD[k * 128:(k + 1) * 128, :],
                                  in_=outS[:])

    nc.compile()
    return nc


# --------------------------------------------------------------------------
# Entry point
# --------------------------------------------------------------------------

def _assemble(results, per_core):
    out = np.zeros((N_NODES, 512), np.float32)
    for c in range(NCORES):
        ob = results[c]["outb"]
        for k, (s, e, _ec) in enumerate(per_core[c]["chunks"]):
            w = e - s
            if w == 0:
                continue
            rows = ob[k * 128: k * 128 + w]
            out[s:e, :C] = rows[:, :C]
            out[s:e, C:] = np.stack(
                [rows[:, C:2 * C], rows[:, 2 * C:3 * C], rows[:, 3 * C:]],
                axis=2).reshape(w, 3 * C)
    return out


def run(inputs, trace=False, **kwargs):
    from concourse.bass_utils import run_bass_kernel_spmd

    per_core = _host_prep(inputs["node_feats"], inputs["edge_attrs"],
                          inputs["edge_feats"], inputs["edge_index"])
    wts = _weights_prep(inputs["W_up_s"], inputs["W_up_v"], inputs["W_mlp1"],
                        inputs["W_mlp2"], inputs["W_mlp3"], inputs["W_mlp4"],
                        inputs["W_lin_s"], inputs["W_lin_v"])
    in_maps = [
        {"gfeat": pc["gfeat"], "hmat": pc["hmat"], "eft": pc["eft"], **wts}
        for pc in per_core
    ]
    nc = build_program()
    res = run_bass_kernel_spmd(nc, in_maps, core_ids=list(range(NCORES)),
                               trace=trace, **kwargs)
    return _assemble(res.results, per_core), res


def kernel(**inputs):
    return run(inputs)[0]


if __name__ == "__main__":
    # smoke: host prep only
    rng = np.random.default_rng(0)
    ins = {
        "node_feats": rng.standard_normal((N_NODES, 512), np.float32),
        "edge_attrs": rng.standard_normal((E_EDGES, 4), np.float32),
        "edge_feats": rng.standard_normal((E_EDGES, RB), np.float32),
        "edge_index": rng.integers(0, N_NODES, (2, E_EDGES)).astype(np.int32),
    }
    pc = _host_prep(ins["node_feats"], ins["edge_attrs"], ins["edge_feats"],
                    ins["edge_index"])
    for c, d in enumerate(pc):
        used = [ch for ch in d["chunks"] if ch[2] > 0]
        print(f"core {c}: {len(used)} chunks used, "
              f"edges={sum(ch[2] for ch in d['chunks'])}")
